# revision 49
# baseline (speedup 1.0000x reference)
"""Trainium2 Bass kernel for batched attention.

Problem: b=16 batches of softmax(Q K^T / sqrt(128)) V with n=m=2048, d=dv=128,
fp32 inputs/outputs.

Sharding: batch dim across 8 NeuronCores (2 batches per core), no comms.

Per-core algorithm (per batch):
  1. Load Q, K with fp32->fp16 cast on DMA (SWDGE), transpose via PE
     (identity matmul, fp16) to get Q^T, K^T in SBUF with d on partitions.
  2. MM1: S^T[mtile, n] = (K^T chunk)-stationary x Q^T-moving in fp16,
     fp32 PSUM accumulate.
  3. exp on ScalarE with fused temperature scale, PSUM->SBUF, fp16 P^T.
  4. MM2: O[ntile, 129] accumulated over m chunks; stationary P^T chunk,
     moving [V | ones] fp16; column 128 is the softmax denominator.
  5. DVE reciprocal + per-partition scale to fp32, store O naturally.

Error vs fp32 reference ~ 5e-4 (fp16 P quantization dominates; exp itself
needs no max-subtraction since scores/temp are ~N(0,1), max ~5.5).
"""

import numpy as np

B = 16
N_CORES = 8
B_LOC = B // N_CORES  # 2 batches per core
N = 2048  # queries per batch
M = 2048  # keys per batch
D = 128   # head dim
NT = N // 128  # 16 n-tiles
MT = M // 128  # 16 m-tiles
INV_TEMP = 1.0 / 11.313708498984761  # 1/sqrt(128)

_CACHE = {}


def _build():
    import concourse.bacc as bacc
    import concourse.mybir as mybir
    import concourse.tile as tile
    from concourse.masks import make_identity

    f32 = mybir.dt.float32
    f16 = mybir.dt.float16

    nc = bacc.Bacc("TRN2", target_bir_lowering=False, debug=False,
                   num_devices=N_CORES)
    q_dram = nc.dram_tensor("queries", [B_LOC, N, D], f32, kind="ExternalInput")
    k_dram = nc.dram_tensor("keys", [B_LOC, M, D], f32, kind="ExternalInput")
    v_dram = nc.dram_tensor("values", [B_LOC, M, D], f32, kind="ExternalInput")
    o_dram = nc.dram_tensor("out", [B_LOC, N, D], f32, kind="ExternalOutput")

    with tile.TileContext(nc) as tc:
        with (
            tc.tile_pool(name="const", bufs=1) as const_pool,
            tc.tile_pool(name="nat", bufs=4) as nat_pool,
            tc.tile_pool(name="qT", bufs=2) as qT_pool,
            tc.tile_pool(name="kT", bufs=2) as kT_pool,
            tc.tile_pool(name="vo", bufs=2) as vo_pool,
            tc.tile_pool(name="pT", bufs=26) as pT_pool,
            tc.tile_pool(name="oall", bufs=2) as o_pool,
            tc.tile_pool(name="small", bufs=8) as small_pool,
            tc.tile_pool(name="partA", bufs=18) as partA_pool,
            tc.tile_pool(name="psS", bufs=2, space="PSUM") as psS_pool,
            tc.tile_pool(name="psO", bufs=4, space="PSUM") as psO_pool,
        ):
            psT_pool = psO_pool  # share the four 1-bank slots
            ident = const_pool.tile([128, 128], f16)
            make_identity(nc, ident[:])

            partials = {}  # (batch, t) -> first-half partial O in SBUF

            def mm2_a(pTs, vo, bkey, t):
                """First-half (c=0..7) partial accumulation -> SBUF."""
                psA = psO_pool.tile([128, 129], f32, tag="psO")
                for c in range(8):
                    nc.tensor.matmul(
                        psA[:],
                        pTs[c][:, t * 128:(t + 1) * 128],
                        vo[:, c * 129:(c + 1) * 129],
                        start=(c == 0), stop=(c == 7))
                pa = partA_pool.tile([128, 129], f32, tag="pa")
                partials[(bkey, t)] = pa
                nc.vector.tensor_copy(pa[:], psA[:])

            def mm2_b(pTs, vo, o_all, bkey, t):
                """Second half (c=8..15), merge with partial, normalize."""
                psO = psO_pool.tile([128, 129], f32)
                for c in range(8, MT):
                    nc.tensor.matmul(
                        psO[:],
                        pTs[c][:, t * 128:(t + 1) * 128],
                        vo[:, c * 129:(c + 1) * 129],
                        start=(c == 8), stop=(c == MT - 1))
                osum = small_pool.tile([128, 129], f32, tag="osum")
                nc.vector.tensor_add(osum[:], psO[:], partials[(bkey, t)][:])
                recip = small_pool.tile([128, 1], f32, tag="recip")
                nc.vector.reciprocal(recip[:], osum[:, 128:129])
                nc.vector.tensor_scalar_mul(
                    o_all[:, t * 128:(t + 1) * 128], osum[:, 0:128], recip[:])

            def store_out(b, o_all):
                for g in range(4):
                    cs = slice(g * 4, (g + 1) * 4)
                    nc.sync.dma_start(
                        o_dram[b].rearrange("(c p) d -> p c d", p=128)[:, cs],
                        o_all[:].rearrange("p (c d) -> p c d", d=128)[:, cs])

            prev = None  # (pTs, vo, o_all, b) of the previous batch
            for b in range(B_LOC):
                # ---- load Q, K with cast to fp16 in 4 chunks each so the
                # transposes can start before the whole tensor lands
                q_nat = nat_pool.tile([128, NT * 128], f16, tag="nat")
                k_nat = nat_pool.tile([128, MT * 128], f16, tag="nat")
                # batch 0: Q chunks early (first two exps need all of Q^T,
                # later K chunks aren't read until well into the window)
                order = (("q", 0), ("k", 0), ("q", 1), ("q", 2), ("q", 3),
                         ("k", 1), ("k", 2), ("k", 3)) if b == 0 else                         tuple(x for g in range(4) for x in (("q", g), ("k", g)))
                for which, g in order:
                    cs = slice(g * 4, (g + 1) * 4)
                    dst, srcd = (q_nat, q_dram) if which == "q" else (k_nat, k_dram)
                    nc.gpsimd.dma_start(
                        dst[:].rearrange("p (c d) -> p c d", d=128)[:, cs],
                        srcd[b].rearrange("(c p) d -> p c d", p=128)[:, cs])

                # ---- load V with cast to fp16, interleaved with a ones column
                vo = vo_pool.tile([128, MT * 129], f16)
                nc.gpsimd.dma_start(
                    vo[:].rearrange("p (c w) -> p c w", w=129)[:, :, 0:128],
                    v_dram[b].rearrange("(c p) d -> p c d", p=128))
                nc.vector.memset(
                    vo[:].rearrange("p (c w) -> p c w", w=129)[:, :, 128:129], 1.0)

                # ---- transpose Q, K via PE into [d, seq] layout (fp16)
                qT = qT_pool.tile([128, N], f16)
                kT = kT_pool.tile([128, M], f16)

                def transp(dst, srct, c):
                    pst0 = psT_pool.tile([128, 129], f32, tag="psO")
                    pst = pst0[:, 0:64].bitcast(f16)
                    nc.tensor.transpose(pst, srct[:, c * 128:(c + 1) * 128],
                                        ident[:])
                    nc.vector.tensor_copy(dst[:, c * 128:(c + 1) * 128], pst)

                def mm1_exp(pT, c, h, split=False):
                    psS = psS_pool.tile([128, 1024], f32, tag="psS")
                    for j in range(2):
                        nc.tensor.matmul(
                            psS[:, j * 512:(j + 1) * 512],
                            kT[:, c * 128:(c + 1) * 128],
                            qT[:, h * 1024 + j * 512:h * 1024 + (j + 1) * 512],
                            start=True, stop=True)
                        if split:
                            nc.scalar.activation(
                                pT[:, h * 1024 + j * 512:
                                   h * 1024 + (j + 1) * 512],
                                psS[:, j * 512:(j + 1) * 512],
                                mybir.ActivationFunctionType.Exp,
                                scale=INV_TEMP)
                    if not split:
                        nc.scalar.activation(
                            pT[:, h * 1024:(h + 1) * 1024], psS[:],
                            mybir.ActivationFunctionType.Exp, scale=INV_TEMP)

                # first MM1/exp interleaved into the transpose stream so the
                # exp pipeline starts before the later DMA chunks land
                pTs = []
                pT0 = pT_pool.tile([128, N], f16, tag="pT")
                pTs.append(pT0)
                transp(kT, k_nat, 0)
                for c2 in range(8):
                    transp(qT, q_nat, c2)
                mm1_exp(pT0, 0, 0)
                for c2 in range(8, NT):
                    transp(qT, q_nat, c2)
                mm1_exp(pT0, 0, 1)
                if prev is not None:
                    mm2_b(*prev, t=0)
                for c2 in range(1, MT):
                    transp(kT, k_nat, c2)

                # ---- MM1 (S^T chunks, fp16) + exp -> P^T fp16, with the
                # previous batch's MM2 t-groups interleaved in program order
                # so the PE alternates long same-shape runs.
                for c in range(1, MT):
                    pT = pT_pool.tile([128, N], f16, tag="pT")
                    pTs.append(pT)
                    for h in range(2):
                        mm1_exp(pT, c, h)
                    if prev is not None:
                        mm2_b(*prev, t=c)
                        if c == MT - 1:
                            store_out(prev[3], prev[2])
                    if 7 <= c < MT - 1:
                        # own-batch first-half MM2 chains inside the exp window
                        for k2 in range(2):
                            t_part = (c - 7) * 2 + k2
                            mm2_a(pTs, vo, b, t_part)

                o_all = o_pool.tile([128, NT * 128], f32)
                prev = (pTs, vo, o_all, b)

            # drain the last batch's MM2: second-half chains + merge
            for t in range(NT):
                mm2_b(*prev, t=t)
            store_out(prev[3], prev[2])

    nc.compile()
    return nc


def _get_nc():
    if "nc" not in _CACHE:
        _CACHE["nc"] = _build()
    return _CACHE["nc"]


def _ensure_ntff_hook():
    """concourse's trace path imports antenv.axon_hooks, which this image's
    antenv lacks; register an equivalent shim so tracing (e.g. BASS_TRACE=1)
    works instead of raising ImportError."""
    import sys
    try:
        import antenv.axon_hooks  # noqa: F401
        return
    except ImportError:
        pass
    import types
    mod = types.ModuleType("antenv.axon_hooks")
    hook = [None]
    mod.set_axon_ntff_profile_hook = lambda h: hook.__setitem__(0, h)
    mod.get_axon_ntff_profile_hook = lambda: hook[0]
    sys.modules["antenv.axon_hooks"] = mod
    try:
        from trn_agent_boot.trn_boot import _ntff_profile_via_ctypes
        mod.set_axon_ntff_profile_hook(
            _ntff_profile_via_ctypes("/opt/axon/libaxon_pjrt.so"))
    except Exception:
        pass


def run(queries, keys, values, trace=False, tmpdir=None):
    """Run on 8 cores; returns (output, BassKernelResults)."""
    _ensure_ntff_hook()
    from concourse.bass_utils import run_bass_kernel_spmd

    nc = _get_nc()
    queries = np.ascontiguousarray(queries, dtype=np.float32)
    keys = np.ascontiguousarray(keys, dtype=np.float32)
    values = np.ascontiguousarray(values, dtype=np.float32)
    in_maps = []
    for c in range(N_CORES):
        s = slice(c * B_LOC, (c + 1) * B_LOC)
        in_maps.append({
            "queries": queries[s],
            "keys": keys[s],
            "values": values[s],
        })
    res = run_bass_kernel_spmd(nc, in_maps, core_ids=list(range(N_CORES)),
                               trace=trace, tmpdir=tmpdir)
    out = np.concatenate([res.results[c]["out"] for c in range(N_CORES)], axis=0)
    return out, res


def kernel(queries, keys, values):
    out, _ = run(queries, keys, values)
    return out


# revision 50
# speedup vs baseline: 1.0048x; 1.0048x over previous
"""Trainium2 Bass kernel for batched attention.

Problem: b=16 batches of softmax(Q K^T / sqrt(128)) V with n=m=2048, d=dv=128,
fp32 inputs/outputs.

Sharding: batch dim across 8 NeuronCores (2 batches per core), no comms.

Per-core algorithm (per batch):
  1. Load Q, K with fp32->fp16 cast on DMA (SWDGE), transpose via PE
     (identity matmul, fp16) to get Q^T, K^T in SBUF with d on partitions.
  2. MM1: S^T[mtile, n] = (K^T chunk)-stationary x Q^T-moving in fp16,
     fp32 PSUM accumulate.
  3. exp on ScalarE with fused temperature scale, PSUM->SBUF, fp16 P^T.
  4. MM2: O[ntile, 129] accumulated over m chunks; stationary P^T chunk,
     moving [V | ones] fp16; column 128 is the softmax denominator.
  5. DVE reciprocal + per-partition scale to fp32, store O naturally.

Error vs fp32 reference ~ 5e-4 (fp16 P quantization dominates; exp itself
needs no max-subtraction since scores/temp are ~N(0,1), max ~5.5).
"""

import numpy as np

B = 16
N_CORES = 8
B_LOC = B // N_CORES  # 2 batches per core
N = 2048  # queries per batch
M = 2048  # keys per batch
D = 128   # head dim
NT = N // 128  # 16 n-tiles
MT = M // 128  # 16 m-tiles
INV_TEMP = 1.0 / 11.313708498984761  # 1/sqrt(128)

_CACHE = {}


def _build():
    import concourse.bacc as bacc
    import concourse.mybir as mybir
    import concourse.tile as tile
    from concourse.masks import make_identity

    f32 = mybir.dt.float32
    f16 = mybir.dt.float16

    nc = bacc.Bacc("TRN2", target_bir_lowering=False, debug=False,
                   num_devices=N_CORES)
    q_dram = nc.dram_tensor("queries", [B_LOC, N, D], f32, kind="ExternalInput")
    k_dram = nc.dram_tensor("keys", [B_LOC, M, D], f32, kind="ExternalInput")
    v_dram = nc.dram_tensor("values", [B_LOC, M, D], f32, kind="ExternalInput")
    o_dram = nc.dram_tensor("out", [B_LOC, N, D], f32, kind="ExternalOutput")

    with tile.TileContext(nc) as tc:
        with (
            tc.tile_pool(name="const", bufs=1) as const_pool,
            tc.tile_pool(name="nat", bufs=3) as nat_pool,
            tc.tile_pool(name="qT", bufs=2) as qT_pool,
            tc.tile_pool(name="kT", bufs=2) as kT_pool,
            tc.tile_pool(name="vo", bufs=2) as vo_pool,
            tc.tile_pool(name="pT", bufs=26) as pT_pool,
            tc.tile_pool(name="oall", bufs=2) as o_pool,
            tc.tile_pool(name="small", bufs=8) as small_pool,
            tc.tile_pool(name="partA", bufs=18) as partA_pool,
            tc.tile_pool(name="psS", bufs=2, space="PSUM") as psS_pool,
            tc.tile_pool(name="psO", bufs=4, space="PSUM") as psO_pool,
        ):
            psT_pool = psO_pool  # share the four 1-bank slots
            ident = const_pool.tile([128, 128], f16)
            make_identity(nc, ident[:])

            partials = {}  # (batch, t) -> first-half partial O in SBUF

            def mm2_a(pTs, vo, bkey, t):
                """First-half (c=0..7) partial accumulation -> SBUF."""
                psA = psO_pool.tile([128, 129], f32, tag="psO")
                for c in range(8):
                    nc.tensor.matmul(
                        psA[:],
                        pTs[c][:, t * 128:(t + 1) * 128],
                        vo[:, c * 129:(c + 1) * 129],
                        start=(c == 0), stop=(c == 7))
                pa = partA_pool.tile([128, 129], f32, tag="pa")
                partials[(bkey, t)] = pa
                nc.vector.tensor_copy(pa[:], psA[:])

            def mm2_b(pTs, vo, o_all, bkey, t):
                """Second half (c=8..15), merge with partial, normalize."""
                psO = psO_pool.tile([128, 129], f32)
                for c in range(8, MT):
                    nc.tensor.matmul(
                        psO[:],
                        pTs[c][:, t * 128:(t + 1) * 128],
                        vo[:, c * 129:(c + 1) * 129],
                        start=(c == 8), stop=(c == MT - 1))
                osum = small_pool.tile([128, 129], f32, tag="osum")
                nc.vector.tensor_add(osum[:], psO[:], partials[(bkey, t)][:])
                recip = small_pool.tile([128, 1], f32, tag="recip")
                nc.vector.reciprocal(recip[:], osum[:, 128:129])
                nc.vector.tensor_scalar_mul(
                    o_all[:, t * 128:(t + 1) * 128], osum[:, 0:128], recip[:])

            def store_out(b, o_all):
                for g in range(4):
                    cs = slice(g * 4, (g + 1) * 4)
                    nc.sync.dma_start(
                        o_dram[b].rearrange("(c p) d -> p c d", p=128)[:, cs],
                        o_all[:].rearrange("p (c d) -> p c d", d=128)[:, cs])

            prev = None  # (pTs, vo, o_all, b) of the previous batch
            for b in range(B_LOC):
                # ---- load Q, K with cast to fp16 in 4 chunks each so the
                # transposes can start before the whole tensor lands
                q_nat = nat_pool.tile([128, NT * 128], f16, tag="nat")
                k_nat = nat_pool.tile([128, MT * 128], f16, tag="nat")
                # batch 0: Q chunks early (first two exps need all of Q^T,
                # later K chunks aren't read until well into the window)
                order = (("q", 0), ("k", 0), ("q", 1), ("q", 2), ("q", 3),
                         ("k", 1), ("k", 2), ("k", 3)) if b == 0 else                         tuple(x for g in range(4) for x in (("q", g), ("k", g)))
                for which, g in order:
                    cs = slice(g * 4, (g + 1) * 4)
                    dst, srcd = (q_nat, q_dram) if which == "q" else (k_nat, k_dram)
                    nc.gpsimd.dma_start(
                        dst[:].rearrange("p (c d) -> p c d", d=128)[:, cs],
                        srcd[b].rearrange("(c p) d -> p c d", p=128)[:, cs])

                # ---- load V with cast to fp16, interleaved with a ones column
                vo = vo_pool.tile([128, MT * 129], f16)
                nc.gpsimd.dma_start(
                    vo[:].rearrange("p (c w) -> p c w", w=129)[:, :, 0:128],
                    v_dram[b].rearrange("(c p) d -> p c d", p=128))
                nc.vector.memset(
                    vo[:].rearrange("p (c w) -> p c w", w=129)[:, :, 128:129], 1.0)

                # ---- transpose Q, K via PE into [d, seq] layout (fp16)
                qT = qT_pool.tile([128, N], f16)
                kT = kT_pool.tile([128, M], f16)

                def transp(dst, srct, c):
                    pst0 = psT_pool.tile([128, 129], f32, tag="psO")
                    pst = pst0[:, 0:64].bitcast(f16)
                    nc.tensor.transpose(pst, srct[:, c * 128:(c + 1) * 128],
                                        ident[:])
                    nc.vector.tensor_copy(dst[:, c * 128:(c + 1) * 128], pst)

                def mm1_exp(pT, c, h, split=False):
                    psS = psS_pool.tile([128, 1024], f32, tag="psS")
                    for j in range(2):
                        nc.tensor.matmul(
                            psS[:, j * 512:(j + 1) * 512],
                            kT[:, c * 128:(c + 1) * 128],
                            qT[:, h * 1024 + j * 512:h * 1024 + (j + 1) * 512],
                            start=True, stop=True)
                        if split:
                            nc.scalar.activation(
                                pT[:, h * 1024 + j * 512:
                                   h * 1024 + (j + 1) * 512],
                                psS[:, j * 512:(j + 1) * 512],
                                mybir.ActivationFunctionType.Exp,
                                scale=INV_TEMP)
                    if not split:
                        nc.scalar.activation(
                            pT[:, h * 1024:(h + 1) * 1024], psS[:],
                            mybir.ActivationFunctionType.Exp, scale=INV_TEMP)

                # first MM1/exp interleaved into the transpose stream so the
                # exp pipeline starts before the later DMA chunks land
                pTs = []
                pT0 = pT_pool.tile([128, N], f16, tag="pT")
                pTs.append(pT0)
                transp(kT, k_nat, 0)
                for c2 in range(8):
                    transp(qT, q_nat, c2)
                mm1_exp(pT0, 0, 0)
                for c2 in range(8, NT):
                    transp(qT, q_nat, c2)
                mm1_exp(pT0, 0, 1)
                if prev is not None:
                    mm2_b(*prev, t=0)
                for c2 in range(1, MT):
                    transp(kT, k_nat, c2)

                # ---- MM1 (S^T chunks, fp16) + exp -> P^T fp16, with the
                # previous batch's MM2 t-groups interleaved in program order
                # so the PE alternates long same-shape runs.
                for c in range(1, MT):
                    pT = pT_pool.tile([128, N], f16, tag="pT")
                    pTs.append(pT)
                    for h in range(2):
                        mm1_exp(pT, c, h)
                    if prev is not None:
                        mm2_b(*prev, t=c)
                        if c == MT - 1:
                            store_out(prev[3], prev[2])
                    if 7 <= c < MT - 1:
                        # own-batch first-half MM2 chains inside the exp window
                        for k2 in range(2):
                            t_part = (c - 7) * 2 + k2
                            mm2_a(pTs, vo, b, t_part)

                o_all = o_pool.tile([128, NT * 128], f32)
                prev = (pTs, vo, o_all, b)

            # drain the last batch's MM2: second-half chains + merge
            for t in range(NT):
                mm2_b(*prev, t=t)
            store_out(prev[3], prev[2])

    nc.compile()
    return nc


def _get_nc():
    if "nc" not in _CACHE:
        _CACHE["nc"] = _build()
    return _CACHE["nc"]


def _ensure_ntff_hook():
    """concourse's trace path imports antenv.axon_hooks, which this image's
    antenv lacks; register an equivalent shim so tracing (e.g. BASS_TRACE=1)
    works instead of raising ImportError."""
    import sys
    try:
        import antenv.axon_hooks  # noqa: F401
        return
    except ImportError:
        pass
    import types
    mod = types.ModuleType("antenv.axon_hooks")
    hook = [None]
    mod.set_axon_ntff_profile_hook = lambda h: hook.__setitem__(0, h)
    mod.get_axon_ntff_profile_hook = lambda: hook[0]
    sys.modules["antenv.axon_hooks"] = mod
    try:
        from trn_agent_boot.trn_boot import _ntff_profile_via_ctypes
        mod.set_axon_ntff_profile_hook(
            _ntff_profile_via_ctypes("/opt/axon/libaxon_pjrt.so"))
    except Exception:
        pass


def run(queries, keys, values, trace=False, tmpdir=None):
    """Run on 8 cores; returns (output, BassKernelResults)."""
    _ensure_ntff_hook()
    from concourse.bass_utils import run_bass_kernel_spmd

    nc = _get_nc()
    queries = np.ascontiguousarray(queries, dtype=np.float32)
    keys = np.ascontiguousarray(keys, dtype=np.float32)
    values = np.ascontiguousarray(values, dtype=np.float32)
    in_maps = []
    for c in range(N_CORES):
        s = slice(c * B_LOC, (c + 1) * B_LOC)
        in_maps.append({
            "queries": queries[s],
            "keys": keys[s],
            "values": values[s],
        })
    res = run_bass_kernel_spmd(nc, in_maps, core_ids=list(range(N_CORES)),
                               trace=trace, tmpdir=tmpdir)
    out = np.concatenate([res.results[c]["out"] for c in range(N_CORES)], axis=0)
    return out, res


def kernel(queries, keys, values):
    out, _ = run(queries, keys, values)
    return out
